# revision 42
# baseline (speedup 1.0000x reference)
"""Trainium2 Bass kernel: causal MHSA, last-position output (bf16, xT-primary).

The reference returns only out[:, -1, :]; with the causal mask the last query
row attends to everything, so per batch element the whole MHSA collapses to
tiny GEMVs (q_row and M = Wk-contracted-with-q fold on the host, removing the
Wq/Wk transfers and the x@Wq/Wk matmuls entirely).  Sharding: pure data
parallel over batch, core b <- batch b, no collectives.

Pipeline (16998ns fp32 baseline -> 12125ns):
  * everything streams as bf16 (host-cast): x DMA halves to ~2.9us and all
    PE ops run 1 cyc/row.  PSUM accumulation stays fp32, so scores, softmax
    sums, attention and the output projection accumulate at full precision;
    measured rel err 4.4e-3 vs the 2e-2 gate.
  * x is sent TRANSPOSED (xT, [f-part, s]): the scores matmuls and softmax
    exps fire straight off the DMA stream with no on-chip dependencies.  The
    PE transpose + PSUM->SBUF copy pipeline regenerates the x-layout, which
    only the end-of-kernel attention matmuls consume, so those copies
    tolerate queueing behind the DMA stream.
  * M rides in front of the first x chunk; Wv/bo/Abd/Bbd and Wo pack into
    two more DMAs (HWDGE descriptor gens are 625ns each, serialized).
  * every softmax exp is emitted right after its scores so the ACT queue
    runs exps ahead of later x-layout copies; copies run in 4-tile groups
    (one PSUM-access fixed cost per group) mostly on DVE.
  * all attention matmuls sit after the last exp (emitting any earlier
    head-of-line-blocks later scores on the in-order PE).
  * the attn^T extraction uses plain strided copies (no reciprocal
    dependency) and one multiply against the block-diag reciprocal pattern
    still in PSUM; PSUM accumulators never share a bank with a concurrent
    accumulation group (hw accumulate state is per-bank).
"""

import numpy as np
from contextlib import ExitStack

import ml_dtypes

import concourse.bass as bass
import concourse.tile as tile
from concourse import bacc, mybir
from concourse.bass_utils import run_bass_kernel_spmd
from concourse.masks import make_identity

B, S, F, PROJ, H, D = 8, 2048, 256, 512, 8, 64
NT = S // 128        # 16 s-tiles
FC = F // 128        # 2 f-chunks
f32 = mybir.dt.float32
bf16 = mybir.dt.bfloat16
EXP = mybir.ActivationFunctionType.Exp
COPY = mybir.ActivationFunctionType.Copy

# xm dram layout: [128, XCOLS] = M (FC*H cols) then xT of every tile
MCOLS = FC * H                    # 16
XCOLS = MCOLS + NT * F            # 4112 (M, then xT of every tile)
# wp dram layout: [Wv | bo | Abd | Bbd] (early DMA) then [Wo] (late DMA)
WV0 = 0
BO0 = WV0 + FC * PROJ             # 1024
ABD0 = BO0 + FC                   # 1026
BBD0 = ABD0 + 128                 # 1154
WO0 = BBD0 + 4                    # 1158
WCOLS = WO0 + 4 * F               # 2182

# x DMA chunks in tiles; first chunk also carries M
XCHUNKS = [(0, 4), (4, 4), (8, 4), (12, 2), (14, 2)]
# ALL tiles arrive transposed (xT): scores and the softmax exps fire straight
# off the DMA stream with no dependencies.  The on-chip transpose+copy
# pipeline regenerates the x-layout, consumed only by the attention matmuls
# at the very end, so those copies tolerate queueing.
CG = [(0, 4), (4, 4), (8, 4), (12, 4)]
# engine for each x-layout copy (True = ACT); ACT also runs the exps
COPY_ON_ACT = [False, True, False, False]
# exp groups (start_tile, ntiles): one ACT activation per entry, ordered so
# the late-arriving tiles aren't queued behind early ones
EG = [(0, 4), (4, 4), (8, 4), (12, 4)]

_cache = {}


def _build():
    nc = bacc.Bacc("TRN2", target_bir_lowering=False, debug=False, num_devices=B)
    xm = nc.dram_tensor("xm", [128, XCOLS], bf16, kind="ExternalInput").ap()
    wp = nc.dram_tensor("wp", [128, WCOLS], bf16, kind="ExternalInput").ap()
    out = nc.dram_tensor("out", [F], f32, kind="ExternalOutput").ap()

    with tile.TileContext(nc) as tc, ExitStack() as ctx:
        P = ctx.enter_context(tc.tile_pool(name="persist", bufs=1))
        xtp = ctx.enter_context(tc.tile_pool(name="xtp", bufs=3, space="PSUM"))
        sct = ctx.enter_context(tc.tile_pool(name="sct", bufs=2, space="PSUM"))
        pers = ctx.enter_context(tc.tile_pool(name="pers", bufs=1, space="PSUM"))
        tailp = ctx.enter_context(tc.tile_pool(name="tailp", bufs=1, space="PSUM"))

        ident = P.tile([128, 128], bf16)
        ones_col = P.tile([128, 1], bf16)
        xm_sb = P.tile([128, XCOLS], bf16)
        x_sb = P.tile([128, NT, F], bf16)
        wp_sb = P.tile([128, WCOLS], bf16)
        wt_sb = P.tile([128, NT * H], bf16)
        srecip = P.tile([H, 1], f32)
        bw_sb = P.tile([H, 4], bf16)
        acr_sb = P.tile([128, 4], f32)
        axT_sb = P.tile([128, FC * H], bf16)
        ac_sb = P.tile([128, 4], bf16)
        o_sb = P.tile([128, FC], f32)
        bo_f32 = P.tile([128, FC], f32)
        dummy = P.tile([1, 1], f32)

        def xT_host(t, c):        # host-transposed tile t, chunk c
            lo = MCOLS + (t * FC + c) * 128
            return xm_sb[:, lo : lo + 128]

        def attn_lhsT(t, c):      # x-layout operand for the attn matmul
            return x_sb[:, t, c * 128 : (c + 1) * 128]

        def mview(c):             # M chunk c  [f-part 128, H]
            return xm_sb[:, c * H : (c + 1) * H]

        def wv(c, pc):            # Wv f-chunk c, proj-chunk pc [128, 128]
            lo = WV0 + c * PROJ + pc * 128
            return wp_sb[:, lo : lo + 128]

        def wo(c, mc):            # Wo proj-chunk c, f-chunk mc [128, 128]
            lo = WO0 + c * F + mc * 128
            return wp_sb[:, lo : lo + 128]

        # trigger the ACT Exp table load early, overlapped with DMA
        nc.vector.memset(dummy[:], 0.0)
        nc.scalar.activation(out=dummy[:], in_=dummy[:], func=EXP)
        nc.vector.memset(ones_col[:], 1.0)
        make_identity(nc, ident[:])

        # ---- DMAs: M rides in front of the first x chunk; packed weights
        #      follow x (needed only in the tail)
        for t0, ntl in XCHUNKS:
            lo = 0 if t0 == 0 else MCOLS + t0 * F
            hi = MCOLS + (t0 + ntl) * F
            nc.sync.dma_start(out=xm_sb[:, lo:hi], in_=xm[:, lo:hi])
        nc.sync.dma_start(out=wp_sb[:, 0:WO0], in_=wp[:, 0:WO0])
        nc.sync.dma_start(out=wp_sb[:, WO0:WCOLS], in_=wp[:, WO0:WCOLS])

        # ---- PE warm-up: hold the p-state ramp open while DMA streams
        warm_ps = xtp.tile([128, FC * 4 * 128], bf16, tag="xt")
        for j in range(20):
            nc.tensor.transpose(warm_ps[:, 0:128], ident[:], ident[:])

        # persistent PSUM accumulators.  axc0/axc1 accumulate concurrently so
        # they need a bank each; sums shares the tail bank (its accumulation
        # window ends before bd/o start, and tile-granularity deps order them)
        tail_ps = tailp.tile([128, 4 + FC + 1], f32, tag="tail")
        bd_ps = tail_ps[:, 0:4]
        o_ps = tail_ps[:, 4 : 4 + FC]
        sums_ps = tail_ps[0:H, 4 + FC : 4 + FC + 1]
        axc_ps = [
            pers.tile([128, H], f32, tag=f"axc{c}", name=f"axc_ps{c}") for c in range(FC)
        ]

        # ---- software-pipelined emission.  PE order: transposes and scores
        #      interleave as data arrives; ALL attention matmuls go last (they
        #      are ~150ns of PE work but each waits on its exp, so putting any
        #      of them mid-stream head-of-line-blocks later scores).
        # sct tiles per exp-group (2 rotating banks -> exps fire as soon as
        # their own group's scores land, not after ALL scores)
        sct_tiles = {}
        for t0, ntl in EG:
            sct_tiles[t0] = sct.tile(
                [128, 4 * H], f32, tag="sc", name=f"sct_ps_{t0}"
            )

        def sct_slice(t_idx):
            for t0, ntl in EG:
                if t0 <= t_idx < t0 + ntl:
                    return sct_tiles[t0][:, (t_idx - t0) * H : (t_idx - t0 + 1) * H]
            raise AssertionError

        xt_tiles = {}

        def emit_T(g):
            t0, ntl = CG[g]
            xt_ps = xtp.tile([128, FC * 4 * 128], bf16, tag="xt", name=f"xt_ps_{g}")
            xt_tiles[g] = xt_ps
            for j in range(ntl):
                for c in range(FC):
                    nc.tensor.transpose(
                        xt_ps[:, (j * FC + c) * 128 : (j * FC + c + 1) * 128],
                        xT_host(t0 + j, c),
                        ident[:],
                    )

        def emit_copy(g):
            t0, ntl = CG[g]
            xt_ps = xt_tiles[g]
            dst = x_sb[:, t0 : t0 + ntl, :]
            srcv = xt_ps[:, 0 : FC * ntl * 128].rearrange(
                "p (j n) -> p j n", j=ntl
            )
            if COPY_ON_ACT[g]:
                nc.scalar.activation(out=dst, in_=srcv, func=COPY)
            else:
                nc.vector.tensor_copy(dst, srcv)

        def emit_scores(g):
            t0, ntl = CG[g]
            for j in range(ntl):
                for c in range(FC):
                    nc.tensor.matmul(
                        sct_slice(t0 + j),
                        xT_host(t0 + j, c),
                        mview(c),
                        start=(c == 0),
                        stop=(c == FC - 1),
                    )

        def emit_exp(t0, ntl):
            nc.scalar.activation(
                out=wt_sb[:, t0 * H : (t0 + ntl) * H],
                in_=sct_tiles[t0][:, 0 : ntl * H],
                func=EXP,
                scale=0.125,
            )

        ATTN_ORDER = list(range(NT))

        def emit_attn(tiles):
            for t_idx in tiles:
                nc.tensor.matmul(
                    sums_ps[:],
                    wt_sb[:, t_idx * H : (t_idx + 1) * H],
                    ones_col[:],
                    start=(t_idx == ATTN_ORDER[0]),
                    stop=(t_idx == ATTN_ORDER[-1]),
                    skip_group_check=True,
                )
                for c in range(FC):
                    nc.tensor.matmul(
                        axc_ps[c][:],
                        attn_lhsT(t_idx, c),
                        wt_sb[:, t_idx * H : (t_idx + 1) * H],
                        start=(t_idx == ATTN_ORDER[0]),
                        stop=(t_idx == ATTN_ORDER[-1]),
                        skip_group_check=True,
                    )

        NG = len(CG)
        emit_T(0)
        emit_T(1)
        emit_copy(0)
        emit_copy(1)
        for g in range(NG):
            if g + 2 < NG:
                emit_T(g + 2)
            emit_scores(g)
            # exps emitted right after their last scores group so they sit
            # ahead of later copies in the ACT queue
            for t0, ntl in EG:
                if t0 + ntl == CG[g][0] + CG[g][1]:
                    emit_exp(t0, ntl)
            if g + 2 < NG:
                emit_copy(g + 2)
        # attention: tiles 0-11 only wait their own (early) exps; the last
        # four matmul right after the final exp
        emit_attn(ATTN_ORDER[:12])
        emit_attn(ATTN_ORDER[12:])

        # ---- softmax denominator: reciprocal straight off the PSUM column,
        #      then the block-diag recip pattern bd[j, c] = recip[2c + (j>=64)]
        #      via one matmul — runs parallel to the attn^T copies
        nc.vector.reciprocal(srecip[:], sums_ps[:])
        nc.vector.tensor_scalar_mul(bw_sb[:], wp_sb[0:H, BBD0 : BBD0 + 4], srecip[:])
        nc.tensor.matmul(
            bd_ps[:], wp_sb[0:H, ABD0 : ABD0 + 128], bw_sb[:], start=True, stop=True
        )


        # ---- attn^T to SBUF (already in [f-part, h] layout for the Wv matmul)
        nc.scalar.activation(out=axT_sb[:, 0:H], in_=axc_ps[0][:], func=COPY)
        nc.vector.tensor_copy(axT_sb[:, H : 2 * H], axc_ps[1][:])

        # ---- attn_full^T blocks [p-part, h]: afT = Wv_block.T @ axT, N=8
        afT_ps = xtp.tile([128, 4 * H], f32, tag="xt")
        for pc in range(4):
            for c in range(FC):
                nc.tensor.matmul(
                    afT_ps[:, pc * H : (pc + 1) * H],
                    wv(c, pc),
                    axT_sb[:, c * H : (c + 1) * H],
                    start=(c == 0),
                    stop=(c == FC - 1),
                )
        # afT[j, 8pc+h] = attn_f[h, 128pc+j]; extract col 10c + (j>=64) per
        # chunk with plain strided copies (no bd dependency -> they fire right
        # after afT), then one multiply against bd still in PSUM
        top = afT_ps[0:64, 0:1]
        bot = afT_ps[64:128, 1:2]
        nc.vector.tensor_copy(
            acr_sb[0:64, 0:4],
            bass.AP(tensor=top.tensor, offset=top.offset, ap=[top.ap[0], [10, 4]]),
        )
        nc.scalar.activation(
            out=acr_sb[64:128, 0:4],
            in_=bass.AP(tensor=bot.tensor, offset=bot.offset, ap=[bot.ap[0], [10, 4]]),
            func=COPY,
        )
        nc.vector.tensor_mul(ac_sb[:], acr_sb[:], bd_ps[:])

        # ---- out[256] = attn_col.T @ Wo (column layout [128, 2]); bias joins
        #      in the final PSUM->SBUF add
        nc.scalar.activation(
            out=bo_f32[:], in_=wp_sb[:, BO0 : BO0 + FC], func=COPY
        )
        for mc in range(FC):
            for c in range(4):
                nc.tensor.matmul(
                    o_ps[:, mc : mc + 1],
                    wo(c, mc),
                    ac_sb[:, c : c + 1],
                    start=(c == 0),
                    stop=(c == 3),
                    skip_group_check=True,
                )
        nc.vector.tensor_add(o_sb[:], o_ps[:], bo_f32[:])
        nc.sync.dma_start(out=out.rearrange("(c p) -> p c", p=128), in_=o_sb[:])

    nc.compile()
    return nc


def get_nc():
    if "nc" not in _cache:
        _cache["nc"] = _build()
    return _cache["nc"]


def host_prep(inputs: dict) -> list[dict]:
    """Per-core input maps: packed bf16 [M | x] plus shared packed weights."""
    xs = np.asarray(inputs["x"], dtype=np.float32)
    Wq = np.asarray(inputs["Wq"], dtype=np.float32)
    Wk = np.asarray(inputs["Wk"], dtype=np.float32)
    Wv = np.asarray(inputs["Wv"], dtype=np.float32)
    Wo = np.asarray(inputs["Wo"], dtype=np.float32)
    bo = np.asarray(inputs["bo"], dtype=np.float32)

    wpack = np.zeros((128, WCOLS), dtype=np.float32)
    # Wv[c*128+p, n] -> wp[p, c*512+n]
    wpack[:, WV0 : WV0 + FC * PROJ] = (
        Wv.reshape(FC, 128, PROJ).transpose(1, 0, 2).reshape(128, FC * PROJ)
    )
    # Wo[c*128+p, n] -> wp[p, 1024 + c*256+n]
    wpack[:, WO0 : WO0 + 4 * F] = (
        Wo.reshape(4, 128, F).transpose(1, 0, 2).reshape(128, 4 * F)
    )
    wpack[:, BO0 : BO0 + FC] = bo.reshape(FC, 128).T
    j = np.arange(128)
    h = np.arange(H)
    wpack[0:H, ABD0 : ABD0 + 128] = (
        (h[:, None] % 2) == (j[None, :] >= 64)
    ).astype(np.float32)
    wpack[0:H, BBD0 : BBD0 + 4] = (
        (h[:, None] // 2) == np.arange(4)[None, :]
    ).astype(np.float32)
    wpack = np.ascontiguousarray(wpack.astype(ml_dtypes.bfloat16))

    in_maps = []
    for b in range(B):
        q_row = xs[b, -1] @ Wq                                   # [512]
        Mb = (Wk * q_row[None, :]).reshape(F, H, D).sum(-1)      # [256, 8]
        xmp = np.empty((128, XCOLS), dtype=np.float32)
        # M[c*128+p, h] -> xm[p, c*8+h]
        xmp[:, 0:MCOLS] = Mb.reshape(FC, 128, H).transpose(1, 0, 2).reshape(
            128, MCOLS
        )
        # every tile xT-layout: xm[p, 16 + (t*FC+c)*128 + s'] = x[t*128+s', c*128+p]
        xmp[:, MCOLS:] = (
            xs[b]
            .reshape(NT, 128, FC, 128)          # [t, s', c, p]
            .transpose(3, 0, 2, 1)              # [p, t, c, s']
            .reshape(128, NT * F)
        )
        in_maps.append(
            {"xm": np.ascontiguousarray(xmp.astype(ml_dtypes.bfloat16)), "wp": wpack}
        )
    return in_maps


def run_hw(inputs: dict) -> np.ndarray:
    nc = get_nc()
    res = run_bass_kernel_spmd(nc, host_prep(inputs), list(range(B)))
    return np.stack([res.results[b]["out"] for b in range(B)])


def kernel(**inputs) -> np.ndarray:
    return run_hw(inputs)


# revision 49
# speedup vs baseline: 1.0312x; 1.0312x over previous
"""Trainium2 Bass kernel: causal MHSA, last-position output (bf16, xT-primary).

The reference returns only out[:, -1, :]; with the causal mask the last query
row attends to everything, so per batch element the whole MHSA collapses to
tiny GEMVs (q_row and M = Wk-contracted-with-q fold on the host, removing the
Wq/Wk transfers and the x@Wq/Wk matmuls entirely).  Sharding: pure data
parallel over batch, core b <- batch b, no collectives.

Pipeline (16998ns fp32 baseline -> 11758ns):
  * everything streams as bf16 (host-cast): x DMA halves to ~2.9us and all
    PE ops run 1 cyc/row.  PSUM accumulation stays fp32, so scores, softmax
    sums, attention and the output projection accumulate at full precision;
    measured rel err 4.4e-3 vs the 2e-2 gate.
  * x is sent TRANSPOSED (xT, [f-part, s]): the scores matmuls and softmax
    exps fire straight off the DMA stream with no on-chip dependencies.  The
    PE transpose + PSUM->SBUF copy pipeline regenerates the x-layout, which
    only the end-of-kernel attention matmuls consume, so those copies
    tolerate queueing behind the DMA stream.
  * M rides in front of the first x chunk; Wv/bo/Abd/Bbd and Wo pack into
    two more DMAs (HWDGE descriptor gens are 625ns each, serialized).
  * every softmax exp is emitted right after its scores so the ACT queue
    runs exps ahead of later x-layout copies; copies run in 4-tile groups
    (one PSUM-access fixed cost per group) mostly on DVE.
  * all attention matmuls sit after the last exp (emitting any earlier
    head-of-line-blocks later scores on the in-order PE).
  * the attn^T extraction uses plain strided copies (no reciprocal
    dependency) and one multiply against the block-diag reciprocal pattern
    still in PSUM; PSUM accumulators never share a bank with a concurrent
    accumulation group (hw accumulate state is per-bank).
  * the framework's four const-tile memsets (all dead here once the EXP bias
    comes from our own zcol tile) are suppressed at module build, releasing
    the all-engine start barrier ~370ns earlier.
"""

import numpy as np
from contextlib import ExitStack

import ml_dtypes

import concourse.bass as bass
import concourse.tile as tile
from concourse import bacc, mybir
from concourse.bass_utils import run_bass_kernel_spmd
from concourse.masks import make_identity

B, S, F, PROJ, H, D = 8, 2048, 256, 512, 8, 64
NT = S // 128        # 16 s-tiles
FC = F // 128        # 2 f-chunks
f32 = mybir.dt.float32
bf16 = mybir.dt.bfloat16
EXP = mybir.ActivationFunctionType.Exp
COPY = mybir.ActivationFunctionType.Copy

# xm dram layout: [128, XCOLS] = M (FC*H cols) then xT of every tile
MCOLS = FC * H                    # 16
XCOLS = MCOLS + NT * F            # 4112 (M, then xT of every tile)
# wp dram layout: [Wv | bo | Abd | Bbd] (early DMA) then [Wo] (late DMA)
WV0 = 0
BO0 = WV0 + FC * PROJ             # 1024
ABD0 = BO0 + FC                   # 1026
BBD0 = ABD0 + 128                 # 1154
WO0 = BBD0 + 4                    # 1158
WCOLS = WO0 + 4 * F               # 2182

# x DMA chunks in tiles; first chunk also carries M
XCHUNKS = [(0, 4), (4, 4), (8, 4), (12, 2), (14, 2)]
# ALL tiles arrive transposed (xT): scores and the softmax exps fire straight
# off the DMA stream with no dependencies.  The on-chip transpose+copy
# pipeline regenerates the x-layout, consumed only by the attention matmuls
# at the very end, so those copies tolerate queueing.
CG = [(0, 4), (4, 4), (8, 4), (12, 4)]
# engine for each x-layout copy (True = ACT); ACT also runs the exps
COPY_ON_ACT = [False, True, False, False]
# exp groups (start_tile, ntiles): one ACT activation per entry, ordered so
# the late-arriving tiles aren't queued behind early ones
EG = [(0, 4), (4, 4), (8, 4), (12, 4)]

_cache = {}

# Bacc.__init__ materializes four const tiles (gpsimd memsets) ahead of the
# all-engine start barrier.  Three of them (f32-1.0, bf16-1.0, u8-127) are
# never read by this program (the BIR verifier flags them as reader-less),
# yet their serialized Pool memsets delay the barrier -- and therefore the
# first x DMA -- by ~300ns.  Suppress emitting just those dead memsets while
# constructing the module; the tiles stay allocated, merely unwritten.
_DEAD_CONSTS = (
    "const-float32-0.0",   # EXP bias: replaced by our own zcol tile
    "const-float32-1.0",
    "const-bfloat16-1.0",
    "const-uint8-127",
)


def _make_bacc():
    owner = None
    for klass in type(bacc.Bacc("TRN2", target_bir_lowering=False,
                                debug=False, num_devices=1).gpsimd).__mro__:
        if "memset" in vars(klass):
            owner = klass
            break
    orig = owner.memset

    def memset(self, ap, constant):
        name = str(getattr(getattr(ap, "tensor", None), "name", ""))
        if any(d in name for d in _DEAD_CONSTS):
            return None
        return orig(self, ap, constant)

    owner.memset = memset
    try:
        return bacc.Bacc("TRN2", target_bir_lowering=False, debug=False,
                         num_devices=B)
    finally:
        owner.memset = orig


def _build():
    nc = _make_bacc()
    xm = nc.dram_tensor("xm", [128, XCOLS], bf16, kind="ExternalInput").ap()
    wp = nc.dram_tensor("wp", [128, WCOLS], bf16, kind="ExternalInput").ap()
    out = nc.dram_tensor("out", [F], f32, kind="ExternalOutput").ap()

    with tile.TileContext(nc) as tc, ExitStack() as ctx:
        P = ctx.enter_context(tc.tile_pool(name="persist", bufs=1))
        xtp = ctx.enter_context(tc.tile_pool(name="xtp", bufs=3, space="PSUM"))
        sct = ctx.enter_context(tc.tile_pool(name="sct", bufs=2, space="PSUM"))
        pers = ctx.enter_context(tc.tile_pool(name="pers", bufs=1, space="PSUM"))
        tailp = ctx.enter_context(tc.tile_pool(name="tailp", bufs=1, space="PSUM"))

        ident = P.tile([128, 128], bf16)
        zcol = P.tile([128, 1], f32)
        ones_col = P.tile([128, 1], bf16)
        xm_sb = P.tile([128, XCOLS], bf16)
        x_sb = P.tile([128, NT, F], bf16)
        wp_sb = P.tile([128, WCOLS], bf16)
        wt_sb = P.tile([128, NT * H], bf16)
        srecip = P.tile([H, 1], f32)
        bw_sb = P.tile([H, 4], bf16)
        acr_sb = P.tile([128, 4], f32)
        axT_sb = P.tile([128, FC * H], bf16)
        ac_sb = P.tile([128, 4], bf16)
        o_sb = P.tile([128, FC], f32)
        bo_f32 = P.tile([128, FC], f32)
        dummy = P.tile([1, 1], f32)

        def xT_host(t, c):        # host-transposed tile t, chunk c
            lo = MCOLS + (t * FC + c) * 128
            return xm_sb[:, lo : lo + 128]

        def attn_lhsT(t, c):      # x-layout operand for the attn matmul
            return x_sb[:, t, c * 128 : (c + 1) * 128]

        def mview(c):             # M chunk c  [f-part 128, H]
            return xm_sb[:, c * H : (c + 1) * H]

        def wv(c, pc):            # Wv f-chunk c, proj-chunk pc [128, 128]
            lo = WV0 + c * PROJ + pc * 128
            return wp_sb[:, lo : lo + 128]

        def wo(c, mc):            # Wo proj-chunk c, f-chunk mc [128, 128]
            lo = WO0 + c * F + mc * 128
            return wp_sb[:, lo : lo + 128]

        # trigger the ACT Exp table load early, overlapped with DMA; zcol
        # replaces the framework's const-0.0 bias tile (suppressed above)
        nc.vector.memset(zcol[:], 0.0)
        nc.vector.memset(dummy[:], 0.0)
        nc.scalar.activation(
            out=dummy[:], in_=dummy[:], func=EXP, bias=zcol[0:1, :]
        )
        nc.vector.memset(ones_col[:], 1.0)
        make_identity(nc, ident[:])

        # ---- DMAs: M rides in front of the first x chunk; packed weights
        #      follow x (needed only in the tail)
        for t0, ntl in XCHUNKS:
            lo = 0 if t0 == 0 else MCOLS + t0 * F
            hi = MCOLS + (t0 + ntl) * F
            nc.sync.dma_start(out=xm_sb[:, lo:hi], in_=xm[:, lo:hi])
        nc.sync.dma_start(out=wp_sb[:, 0:WO0], in_=wp[:, 0:WO0])
        nc.sync.dma_start(out=wp_sb[:, WO0:WCOLS], in_=wp[:, WO0:WCOLS])

        # ---- PE warm-up: hold the p-state ramp open while DMA streams
        warm_ps = xtp.tile([128, FC * 4 * 128], bf16, tag="xt")
        for j in range(20):
            nc.tensor.transpose(warm_ps[:, 0:128], ident[:], ident[:])

        # persistent PSUM accumulators.  axc0/axc1 accumulate concurrently so
        # they need a bank each; sums shares the tail bank (its accumulation
        # window ends before bd/o start, and tile-granularity deps order them)
        tail_ps = tailp.tile([128, 4 + FC + 1], f32, tag="tail")
        bd_ps = tail_ps[:, 0:4]
        o_ps = tail_ps[:, 4 : 4 + FC]
        sums_ps = tail_ps[0:H, 4 + FC : 4 + FC + 1]
        axc_ps = [
            pers.tile([128, H], f32, tag=f"axc{c}", name=f"axc_ps{c}") for c in range(FC)
        ]

        # ---- software-pipelined emission.  PE order: transposes and scores
        #      interleave as data arrives; ALL attention matmuls go last (they
        #      are ~150ns of PE work but each waits on its exp, so putting any
        #      of them mid-stream head-of-line-blocks later scores).
        # sct tiles per exp-group (2 rotating banks -> exps fire as soon as
        # their own group's scores land, not after ALL scores)
        sct_tiles = {}
        for t0, ntl in EG:
            sct_tiles[t0] = sct.tile(
                [128, 4 * H], f32, tag="sc", name=f"sct_ps_{t0}"
            )

        def sct_slice(t_idx):
            for t0, ntl in EG:
                if t0 <= t_idx < t0 + ntl:
                    return sct_tiles[t0][:, (t_idx - t0) * H : (t_idx - t0 + 1) * H]
            raise AssertionError

        xt_tiles = {}

        def emit_T(g):
            t0, ntl = CG[g]
            xt_ps = xtp.tile([128, FC * 4 * 128], bf16, tag="xt", name=f"xt_ps_{g}")
            xt_tiles[g] = xt_ps
            for j in range(ntl):
                for c in range(FC):
                    nc.tensor.transpose(
                        xt_ps[:, (j * FC + c) * 128 : (j * FC + c + 1) * 128],
                        xT_host(t0 + j, c),
                        ident[:],
                    )

        def emit_copy(g):
            t0, ntl = CG[g]
            xt_ps = xt_tiles[g]
            dst = x_sb[:, t0 : t0 + ntl, :]
            srcv = xt_ps[:, 0 : FC * ntl * 128].rearrange(
                "p (j n) -> p j n", j=ntl
            )
            if COPY_ON_ACT[g]:
                nc.scalar.activation(out=dst, in_=srcv, func=COPY)
            else:
                nc.vector.tensor_copy(dst, srcv)

        def emit_scores(g):
            t0, ntl = CG[g]
            for j in range(ntl):
                for c in range(FC):
                    nc.tensor.matmul(
                        sct_slice(t0 + j),
                        xT_host(t0 + j, c),
                        mview(c),
                        start=(c == 0),
                        stop=(c == FC - 1),
                    )

        def emit_exp(t0, ntl):
            nc.scalar.activation(
                out=wt_sb[:, t0 * H : (t0 + ntl) * H],
                in_=sct_tiles[t0][:, 0 : ntl * H],
                func=EXP,
                bias=zcol[:],
                scale=0.125,
            )

        ATTN_ORDER = list(range(NT))

        def emit_attn(tiles):
            for t_idx in tiles:
                nc.tensor.matmul(
                    sums_ps[:],
                    wt_sb[:, t_idx * H : (t_idx + 1) * H],
                    ones_col[:],
                    start=(t_idx == ATTN_ORDER[0]),
                    stop=(t_idx == ATTN_ORDER[-1]),
                    skip_group_check=True,
                )
                for c in range(FC):
                    nc.tensor.matmul(
                        axc_ps[c][:],
                        attn_lhsT(t_idx, c),
                        wt_sb[:, t_idx * H : (t_idx + 1) * H],
                        start=(t_idx == ATTN_ORDER[0]),
                        stop=(t_idx == ATTN_ORDER[-1]),
                        skip_group_check=True,
                    )

        NG = len(CG)
        emit_T(0)
        emit_T(1)
        emit_copy(0)
        emit_copy(1)
        for g in range(NG):
            if g + 2 < NG:
                emit_T(g + 2)
            emit_scores(g)
            # exps emitted right after their last scores group so they sit
            # ahead of later copies in the ACT queue
            for t0, ntl in EG:
                if t0 + ntl == CG[g][0] + CG[g][1]:
                    emit_exp(t0, ntl)
            if g + 2 < NG:
                emit_copy(g + 2)
        # attention: tiles 0-11 only wait their own (early) exps; the last
        # four matmul right after the final exp
        emit_attn(ATTN_ORDER[:12])
        emit_attn(ATTN_ORDER[12:])

        # ---- softmax denominator: reciprocal straight off the PSUM column,
        #      then the block-diag recip pattern bd[j, c] = recip[2c + (j>=64)]
        #      via one matmul — runs parallel to the attn^T copies
        nc.vector.reciprocal(srecip[:], sums_ps[:])
        nc.vector.tensor_scalar_mul(bw_sb[:], wp_sb[0:H, BBD0 : BBD0 + 4], srecip[:])
        nc.tensor.matmul(
            bd_ps[:], wp_sb[0:H, ABD0 : ABD0 + 128], bw_sb[:], start=True, stop=True
        )


        # ---- attn^T to SBUF (already in [f-part, h] layout for the Wv matmul)
        nc.scalar.activation(out=axT_sb[:, 0:H], in_=axc_ps[0][:], func=COPY)
        nc.vector.tensor_copy(axT_sb[:, H : 2 * H], axc_ps[1][:])

        # ---- attn_full^T blocks [p-part, h]: afT = Wv_block.T @ axT, N=8
        afT_ps = xtp.tile([128, 4 * H], f32, tag="xt")
        for pc in range(4):
            for c in range(FC):
                nc.tensor.matmul(
                    afT_ps[:, pc * H : (pc + 1) * H],
                    wv(c, pc),
                    axT_sb[:, c * H : (c + 1) * H],
                    start=(c == 0),
                    stop=(c == FC - 1),
                )
        # afT[j, 8pc+h] = attn_f[h, 128pc+j]; extract col 10c + (j>=64) per
        # chunk with plain strided copies (no bd dependency -> they fire right
        # after afT), then one multiply against bd still in PSUM
        top = afT_ps[0:64, 0:1]
        bot = afT_ps[64:128, 1:2]
        nc.vector.tensor_copy(
            acr_sb[0:64, 0:4],
            bass.AP(tensor=top.tensor, offset=top.offset, ap=[top.ap[0], [10, 4]]),
        )
        nc.scalar.activation(
            out=acr_sb[64:128, 0:4],
            in_=bass.AP(tensor=bot.tensor, offset=bot.offset, ap=[bot.ap[0], [10, 4]]),
            func=COPY,
        )
        nc.vector.tensor_mul(ac_sb[:], acr_sb[:], bd_ps[:])

        # ---- out[256] = attn_col.T @ Wo (column layout [128, 2]); bias joins
        #      in the final PSUM->SBUF add
        nc.scalar.activation(
            out=bo_f32[:], in_=wp_sb[:, BO0 : BO0 + FC], func=COPY
        )
        for mc in range(FC):
            for c in range(4):
                nc.tensor.matmul(
                    o_ps[:, mc : mc + 1],
                    wo(c, mc),
                    ac_sb[:, c : c + 1],
                    start=(c == 0),
                    stop=(c == 3),
                    skip_group_check=True,
                )
        nc.vector.tensor_add(o_sb[:], o_ps[:], bo_f32[:])
        nc.sync.dma_start(out=out.rearrange("(c p) -> p c", p=128), in_=o_sb[:])

    nc.compile()
    return nc


def get_nc():
    if "nc" not in _cache:
        _cache["nc"] = _build()
    return _cache["nc"]


def host_prep(inputs: dict) -> list[dict]:
    """Per-core input maps: packed bf16 [M | x] plus shared packed weights."""
    xs = np.asarray(inputs["x"], dtype=np.float32)
    Wq = np.asarray(inputs["Wq"], dtype=np.float32)
    Wk = np.asarray(inputs["Wk"], dtype=np.float32)
    Wv = np.asarray(inputs["Wv"], dtype=np.float32)
    Wo = np.asarray(inputs["Wo"], dtype=np.float32)
    bo = np.asarray(inputs["bo"], dtype=np.float32)

    wpack = np.zeros((128, WCOLS), dtype=np.float32)
    # Wv[c*128+p, n] -> wp[p, c*512+n]
    wpack[:, WV0 : WV0 + FC * PROJ] = (
        Wv.reshape(FC, 128, PROJ).transpose(1, 0, 2).reshape(128, FC * PROJ)
    )
    # Wo[c*128+p, n] -> wp[p, 1024 + c*256+n]
    wpack[:, WO0 : WO0 + 4 * F] = (
        Wo.reshape(4, 128, F).transpose(1, 0, 2).reshape(128, 4 * F)
    )
    wpack[:, BO0 : BO0 + FC] = bo.reshape(FC, 128).T
    j = np.arange(128)
    h = np.arange(H)
    wpack[0:H, ABD0 : ABD0 + 128] = (
        (h[:, None] % 2) == (j[None, :] >= 64)
    ).astype(np.float32)
    wpack[0:H, BBD0 : BBD0 + 4] = (
        (h[:, None] // 2) == np.arange(4)[None, :]
    ).astype(np.float32)
    wpack = np.ascontiguousarray(wpack.astype(ml_dtypes.bfloat16))

    in_maps = []
    for b in range(B):
        q_row = xs[b, -1] @ Wq                                   # [512]
        Mb = (Wk * q_row[None, :]).reshape(F, H, D).sum(-1)      # [256, 8]
        xmp = np.empty((128, XCOLS), dtype=np.float32)
        # M[c*128+p, h] -> xm[p, c*8+h]
        xmp[:, 0:MCOLS] = Mb.reshape(FC, 128, H).transpose(1, 0, 2).reshape(
            128, MCOLS
        )
        # every tile xT-layout: xm[p, 16 + (t*FC+c)*128 + s'] = x[t*128+s', c*128+p]
        xmp[:, MCOLS:] = (
            xs[b]
            .reshape(NT, 128, FC, 128)          # [t, s', c, p]
            .transpose(3, 0, 2, 1)              # [p, t, c, s']
            .reshape(128, NT * F)
        )
        in_maps.append(
            {"xm": np.ascontiguousarray(xmp.astype(ml_dtypes.bfloat16)), "wp": wpack}
        )
    return in_maps


def run_hw(inputs: dict) -> np.ndarray:
    nc = get_nc()
    res = run_bass_kernel_spmd(nc, host_prep(inputs), list(range(B)))
    return np.stack([res.results[b]["out"] for b in range(B)])


def kernel(**inputs) -> np.ndarray:
    return run_hw(inputs)
